# revision 21
# baseline (speedup 1.0000x reference)
"""FFT-block kernel for Trainium2 (8 NeuronCores, batch-data-parallel).

Computation (per sample):
  y0  = mean(x, (H, W))                      [C]
  h   = relu(y0 @ W1c.T + b1)                [C/6]
  y   = sigmoid(h @ W2c.T + b2)              [C]
  s1  = relu(y @ Ws1.T + bs1)                [CF]
  s2  = relu(y @ Ws2.T + bs2)                [CF]
  yf  = rfft(y); amp=|yf|*s1; pha=angle(yf)*s2
  rec = amp*(cos(pha) + i sin(pha)); xr = irfft(rec, C)
  out = (xr * y)[:, None, None]

Strategy: batch dim (16) sharded 2-per-core. The 400MB stream of x feeds a
free-axis reduction; chunks alternate between the DVE reduce and the ACT
accum_out path.  The stream is paced by the 16 DMA engines (~27 GB/s each,
~430 GB/s aggregate), so the tail after the last byte is what matters:

- ACT's stream work is front-loaded so its activation-table loads (sigmoid
  -> sqrt -> arctan -> sin: arctan and sin share trig_and_small, 3 loads
  total) retire before / underneath other work.
- The last row-tile's chunks cascade down to 256 cols so the final reduce
  is short; per-chunk partials land in per-engine column tiles combined by
  one reduce each.
- The MLP/FFT tail runs batch-major: y is computed as a [2, C] row via a
  ones-row affine trick (bias folded into the matmul), PE-transposed into
  [C, 2] columns for the four projections (fp16 weights, 193 moving cols
  into [2, CF] PSUM tiles - bins on the free axis means ONE trig chain,
  not two), and the irfft is 4 fp16 matmuls producing [2, C] directly so
  the output needs no transposes.  All tail matrices are fp16 (bf16's
  0.4% rounding is too coarse for the AC bins of the DFT); w1t is bf16
  because 1/HW folding makes its values fp16-subnormal.
- angle() uses atan2(y,x) = 2*atan(y/(|z|+x)) with the atan argument
  folded into [0,1]; sin/cos use an exact mod-2pi range reduction (fp32
  round-to-int magic).  Nyquist (Im==0 analytically) special-cased via
  sign(Re); DC is provably Re>0 (sum of sigmoids) so it needs no fix.
"""

import numpy as np
from contextlib import ExitStack

import concourse.bass as bass
import concourse.bacc as bacc
import concourse.tile as tile
from concourse import mybir
from concourse.bass_utils import run_bass_kernel_spmd

B, C, H, W = 16, 384, 128, 128
NCORES = 8
BPC = B // NCORES            # 2 samples per core
CH = C // 6                  # 64
CF = C // 2 + 1              # 193 rfft bins
HW = H * W                   # 16384
FP32 = mybir.dt.float32
FP16 = mybir.dt.float16
BF16 = mybir.dt.bfloat16
U16 = mybir.dt.uint16
AF = mybir.ActivationFunctionType
AX = mybir.AxisListType
OP = mybir.AluOpType

F_CHUNK = 4096               # free-dim chunk of the x stream
STREAM_BUFS = 10

KC = [0, 128, 256]           # channel chunks (3 x 128)
FC = [(0, 128), (128, 65)]   # freq-bin chunks (128 + 65)
NYQ = 192                    # Nyquist bin index
MAGIC = 12582912.0           # 1.5 * 2**23: x+MAGIC-MAGIC == round(x) in fp32

# stream chunk schedule: (sizes, engines) per row-tile. 'D' = DVE reduce,
# 'A' = ACT accum.  Strict D/A alternation on the 4096-col chunks: runs of
# consecutive same-engine full chunks stall SBUF ring recycling and the
# DMA engines lose ~15% per-packet throughput for the whole endgame.  The
# cascade keeps the final reduce short; ACT's last chunk (t5/j5) lands
# ~0.6us before the stream ends, leaving room for the sigmoid table load
# in front of y's matmul.
_FULL = [F_CHUNK] * 4
_SCHED = [
    (_FULL, "DADA"),
    (_FULL, "DADA"),
    (_FULL, "DADA"),
    (_FULL, "DADA"),
    (_FULL, "DADA"),
    ([4096, 4096, 4096, 1024, 1024, 1024, 512, 256, 256], "DAADADADD"),
]
assert all(sum(s) == HW for s, _ in _SCHED)

# ---- packed-constant column layout (u16 tensor; regions bitcast to
# bf16/fp16 on device) ----
_OFF = {}
_tot = 0


def _alloc_cols(name, ncols):
    global _tot
    _OFF[name] = _tot
    _tot += ncols


for _k in range(3):
    _alloc_cols(f"w1t{_k}", CH)          # bf16 [128, 64]
_alloc_cols("w2r", C)                    # fp16 [65, 384] (row 64 = b2)
for _nm in ("csm", "w12"):               # packed pairs [cm|sm], [ws1|ws2]
    for _k in range(3):
        _alloc_cols(f"{_nm}{_k}", 2 * CF)  # fp16 [128, 386]
_alloc_cols("bsrow", 2 * CF)             # fp16 [1, 386] = [bs1|bs2]
_alloc_cols("id2h", BPC)                 # fp16 eye(2)
for _j in range(2):
    _alloc_cols(f"icr{_j}", C)           # fp16 [128/65, 384]
    _alloc_cols(f"ici{_j}", C)
_alloc_cols("ones2", BPC)                # fp16 [1, 2]
TOTU = _tot
TOTF = 3                                 # fp32: b1 [64,1], id2 [2,2]


def _build():
    nc = bacc.Bacc(
        "TRN2",
        target_bir_lowering=False,
        debug=False,
        enable_asserts=False,
        num_devices=NCORES,
    )

    xs = nc.dram_tensor("xs", [BPC, C, H, W], FP32, kind="ExternalInput")
    wu = nc.dram_tensor("wu", [128, TOTU], U16, kind="ExternalInput")
    wf = nc.dram_tensor("wf", [128, TOTF], FP32, kind="ExternalInput")
    outp = nc.dram_tensor("out", [BPC, C, 1, 1], FP32, kind="ExternalOutput")

    with tile.TileContext(nc) as tc, ExitStack() as ctx:
        persist = ctx.enter_context(tc.tile_pool(name="persist", bufs=1))
        stream = ctx.enter_context(tc.tile_pool(name="stream", bufs=STREAM_BUFS))
        ps_mm = ctx.enter_context(
            tc.tile_pool(name="ps_mm", bufs=6, space=bass.MemorySpace.PSUM)
        )
        ps_fin = ctx.enter_context(
            tc.tile_pool(name="ps_fin", bufs=1, space=bass.MemorySpace.PSUM)
        )

        # ---- constants in two DMAs on the ACT queue (sync queue runs the
        # x stream exclusively) ----
        wub = persist.tile([128, TOTU], U16, tag="wub", name="wub")
        nc.scalar.dma_start(out=wub, in_=wu[:, :])
        wfb = persist.tile([128, TOTF], FP32, tag="wfb", name="wfb")
        nc.scalar.dma_start(out=wfb, in_=wf[:, :])

        def cs16(name, rows, ncols, dt):
            o = _OFF[name]
            return wub[:rows, o : o + ncols].bitcast(dt)

        w1t_sb = [cs16(f"w1t{k}", 128, CH, BF16) for k in range(3)]
        w2r_sb = cs16("w2r", CH + 1, C, FP16)
        mats = {
            nm: [cs16(f"{nm}{k}", 128, 2 * CF, FP16) for k in range(3)]
            for nm in ("csm", "w12")
        }
        bsrow = cs16("bsrow", 1, 2 * CF, FP16)
        id2h = cs16("id2h", BPC, BPC, FP16)
        icr_sb = [cs16(f"icr{j}", l, C, FP16) for j, (s, l) in enumerate(FC)]
        ici_sb = [cs16(f"ici{j}", l, C, FP16) for j, (s, l) in enumerate(FC)]
        ones2 = cs16("ones2", 1, BPC, FP16)
        b1_sb = wfb[:CH, 0:1]
        id2 = wfb[:BPC, 1 : 1 + BPC]

        def T(tag, shape=(BPC, CF), dt=FP32):
            return persist.tile(list(shape), dt, tag=tag, name=tag)

        # h2 = [h; ones] (fp16) - ones row set once, rows 0-63 written later
        h2 = T("h2", (CH + 1, BPC), FP16)
        nc.vector.memset(h2[CH : CH + 1, :], 1.0)

        # ---- phase 1: stream x, per-(b,c)-row sums over the spatial axis.
        # DVE partials land in columns of Dpart_t, ACT accums in Apart_t;
        # one reduce each + one add finalizes yt as bf16 (matmul rhs).
        xrows = xs.rearrange("b c h w -> (b c) (h w)")
        dummy = persist.tile([128, F_CHUNK], BF16, tag="dummy", name="dummy")
        ytb = [T(f"ytb{k}", (128, BPC), BF16) for k in range(3)]
        ph = [None, None]

        for t, (sizes, engines) in enumerate(_SCHED):
            b, k = divmod(t, 3)
            nD = engines.count("D")
            nA = engines.count("A")
            Dpart = T(f"Dp{t}", (128, nD))
            Apart = T(f"Ap{t}", (128, nA))
            off = jD = jA = 0
            for j, fch in enumerate(sizes):
                chk = stream.tile([128, fch], FP32, tag="stream", name=f"chk{t}_{j}")
                nc.sync.dma_start(
                    out=chk,
                    in_=xrows[t * 128 : (t + 1) * 128, off : off + fch],
                )
                off += fch
                if engines[j] == "D":
                    nc.vector.reduce_sum(out=Dpart[:, jD : jD + 1], in_=chk, axis=AX.X)
                    jD += 1
                else:
                    nc.scalar.activation(
                        out=dummy[:, :fch], in_=chk, func=AF.Identity,
                        accum_out=Apart[:, jA : jA + 1],
                    )
                    jA += 1
            Dsum = T(f"Ds{t}", (128, 1))
            nc.vector.reduce_sum(out=Dsum, in_=Dpart, axis=AX.X)
            Asum = T(f"As{t}", (128, 1))
            nc.vector.reduce_sum(out=Asum, in_=Apart, axis=AX.X)
            nc.vector.tensor_add(out=ytb[k][:, b : b + 1], in0=Dsum, in1=Asum)

            # squeeze matmul contribution for this (b, k) right away; only
            # the (b=1, k=2) one lands after the stream.
            if k == 0:
                ph[b] = ps_mm.tile([CH, 1], FP32, tag="mm", name=f"ph{b}")
            nc.tensor.matmul(
                ph[b], lhsT=w1t_sb[k], rhs=ytb[k][:, b : b + 1],
                start=(k == 0), stop=(k == 2),
            )
            if k == 2:  # h = relu(ph + b1) -> fp16 column of h2
                htmp = T(f"htmp{b}", (CH, 1))
                nc.vector.tensor_add(out=htmp, in0=ph[b], in1=b1_sb)
                nc.vector.tensor_scalar_max(
                    out=h2[:CH, b : b + 1], in0=htmp, scalar1=0.0
                )

        # ---- phase 2: y as a [2, C] row (bias via the ones row) ----
        py = ps_mm.tile([BPC, C], FP32, tag="mm", name="py")
        nc.tensor.matmul(py, lhsT=h2, rhs=w2r_sb, start=True, stop=True)
        y_row = T("y_row", (BPC, C))
        nc.scalar.activation(out=y_row, in_=py, func=AF.Sigmoid)

        # y columns for the projections: PE-transpose 128-col slices
        y_col = []
        for m, s in enumerate(KC):
            yT = ps_mm.tile([128, BPC], FP32, tag="mm", name=f"yT{m}")
            nc.tensor.transpose(yT, y_row[:, s : s + 128], id2)
            yc = T(f"yc{m}", (128, BPC), FP16)
            nc.vector.tensor_copy(out=yc, in_=yT)
            y_col.append(yc)

        # ---- projections, pairwise packed: [2, 386] PSUM tiles hold
        # [re|im] and [s1|s2] (bins on the free axis).  re/im first (they
        # gate the trig chain); s1/s2 bias via a rank-1 ones matmul.
        prem = ps_mm.tile([BPC, 2 * CF], FP32, tag="mm", name="prem")
        for k in range(3):
            nc.tensor.matmul(prem, lhsT=y_col[k], rhs=mats["csm"][k],
                             start=(k == 0), stop=(k == 2))
        ps12 = ps_mm.tile([BPC, 2 * CF], FP32, tag="mm", name="ps12")
        for k in range(3):
            nc.tensor.matmul(ps12, lhsT=y_col[k], rhs=mats["w12"][k],
                             start=(k == 0), stop=False)
        nc.tensor.matmul(ps12, lhsT=ones2, rhs=bsrow, start=False, stop=True)
        pre = prem[:, :CF]
        pim = prem[:, CF:]
        ps1 = ps12[:, :CF]
        ps2 = ps12[:, CF:]

        # ---- trig chain on [2, 193] ----
        # One serial DVE chain; off-chain ops are queued into the windows
        # where DVE would otherwise wait on an ACT result (sqrt/arctan).
        # atan2 via one approximate reciprocal: u = min(|im|, |z|+re) /
        # max(|im|, |z|+re); the |t|>1 fold becomes (|im| > |z|+re) and the
        # sign/fold application collapses to at = a*g1 + g0.
        crem = T("crem", (BPC, 2 * CF))  # one wide PSUM->SBUF copy
        nc.vector.tensor_copy(out=crem, in_=prem)
        cre = crem[:, :CF]
        cim = crem[:, CF:]
        r2 = T("r2")
        nc.vector.tensor_mul(out=r2, in0=cre, in1=pre)
        i2 = T("i2")
        nc.vector.tensor_mul(out=i2, in0=cim, in1=pim)
        nc.vector.tensor_add(out=r2, in0=r2, in1=i2)
        amp0 = T("amp0")
        nc.scalar.activation(out=amp0, in_=r2, func=AF.Sqrt)
        # fillers while ACT runs sqrt:
        absim = T("absim")  # |im| = max(-im, im)
        nc.vector.scalar_tensor_tensor(
            out=absim, in0=cim, scalar=-1.0, in1=cim, op0=OP.mult, op1=OP.max
        )
        sgn = T("sgn")  # 2*(im>0) - 1
        nc.vector.tensor_scalar(
            out=sgn, in0=cim, scalar1=0.0, scalar2=2.0, op0=OP.is_gt, op1=OP.mult
        )
        nc.vector.tensor_scalar_sub(out=sgn, in0=sgn, scalar1=1.0)
        fpn = T("fpn", (BPC, 1))  # Nyquist: Re>0 (Im==0 analytically there)
        nc.vector.tensor_scalar(
            out=fpn, in0=pre[:, NYQ : NYQ + 1], scalar1=0.0, scalar2=None,
            op0=OP.is_gt,
        )
        den0 = T("den0")  # |z| + re >= 0
        nc.vector.tensor_add(out=den0, in0=amp0, in1=pre)
        mx = T("mx")
        nc.vector.tensor_tensor(out=mx, in0=absim, in1=den0, op=OP.max)
        rmx = T("rmx")  # 1/mx, fast approx (~18 bits)
        nc.vector.reciprocal_approx_fast(out=rmx, in_=mx)
        mn = T("mn")
        nc.vector.tensor_tensor(out=mn, in0=absim, in1=den0, op=OP.min)
        u = T("u")
        nc.vector.tensor_mul(out=u, in0=mn, in1=rmx)
        a = T("a")  # atan(u) in [0, pi/4]
        nc.scalar.activation(out=a, in_=u, func=AF.Arctan)
        # fillers while ACT runs arctan: fold masks + s1/s2 post-ops
        fgt = T("fgt")  # |tan(angle/2)| > 1
        nc.vector.tensor_tensor(out=fgt, in0=absim, in1=den0, op=OP.is_gt)
        g1 = T("g1")  # sgn*(1-2*fgt)
        nc.vector.tensor_scalar(
            out=g1, in0=fgt, scalar1=-2.0, scalar2=1.0, op0=OP.mult, op1=OP.add
        )
        nc.vector.tensor_mul(out=g1, in0=g1, in1=sgn)
        g0 = T("g0")  # sgn*(pi/2)*fgt
        nc.vector.scalar_tensor_tensor(
            out=g0, in0=sgn, scalar=float(np.pi / 2), in1=fgt,
            op0=OP.mult, op1=OP.mult,
        )
        s1r = T("s1r")  # relu(ps1)
        nc.vector.tensor_scalar_max(out=s1r, in0=ps1, scalar1=0.0)
        s2s = T("s2s")  # relu(ps2)/pi
        nc.vector.tensor_scalar(
            out=s2s, in0=ps2, scalar1=0.0, scalar2=float(1.0 / np.pi),
            op0=OP.max, op1=OP.mult,
        )
        amp = T("amp")  # |z|*s1
        nc.vector.tensor_mul(out=amp, in0=amp0, in1=s1r)
        at = T("at")  # angle/2 (signed) = a*g1 + g0
        nc.vector.tensor_mul(out=at, in0=a, in1=g1)
        nc.vector.tensor_add(out=at, in0=at, in1=g0)
        # Nyquist: angle is exactly 0 (Re>0) or pi: at = pi/2 * (1 - (Re>0))
        nc.vector.tensor_scalar(
            out=at[:, NYQ : NYQ + 1], in0=fpn,
            scalar1=float(-np.pi / 2), scalar2=float(np.pi / 2),
            op0=OP.mult, op1=OP.add,
        )
        r_ = T("r_")  # pha / 2pi
        nc.vector.tensor_mul(out=r_, in0=at, in1=s2s)
        # sin branch first (irfft consumes ri first), cos follows
        n1 = T("n1")
        nc.vector.tensor_scalar(
            out=n1, in0=r_, scalar1=MAGIC, scalar2=MAGIC, op0=OP.add, op1=OP.subtract
        )
        nc.vector.tensor_sub(out=n1, in0=r_, in1=n1)
        sn = T("sn")
        nc.scalar.activation(out=sn, in_=n1, func=AF.Sin, scale=float(2 * np.pi))
        # cos arg from the sin arg: frac2 = (frac1 + 0.25) - (frac1 > 0.25)
        # stays in [-0.5, 0.5] (fillers while ACT runs the first sin)
        q4 = T("q4")
        nc.vector.tensor_scalar(
            out=q4, in0=n1, scalar1=0.25, scalar2=None, op0=OP.is_gt
        )
        n2 = T("n2")
        nc.vector.scalar_tensor_tensor(
            out=n2, in0=n1, scalar=0.25, in1=q4, op0=OP.add, op1=OP.subtract
        )
        cs = T("cs")
        nc.scalar.activation(out=cs, in_=n2, func=AF.Sin, scale=float(2 * np.pi))
        ri = T("ri", dt=FP16)
        nc.vector.tensor_mul(out=ri, in0=amp, in1=sn)
        rr = T("rr", dt=FP16)
        nc.vector.tensor_mul(out=rr, in0=amp, in1=cs)

        # ---- irfft as 4 fp16 matmuls into [2, C]; rec transposed into
        # [bins, 2] fp16 columns first ----
        recb = {}
        for nm, src in (("ri", ri), ("rr", rr)):
            for j, (s, l) in enumerate(FC):
                rT = ps_mm.tile([l, BPC], FP16, tag="mm", name=f"{nm}T{j}")
                nc.tensor.transpose(rT, src[:, s : s + l], id2h)
                rb = T(f"{nm}b{j}", (l, BPC), FP16)
                nc.vector.tensor_copy(out=rb, in_=rT)
                recb[(nm, j)] = rb
        pfin = ps_fin.tile([BPC, C], FP32, tag="pfin", name="pfin")
        steps = [
            (recb[("ri", 0)], ici_sb[0]), (recb[("rr", 0)], icr_sb[0]),
            (recb[("ri", 1)], ici_sb[1]), (recb[("rr", 1)], icr_sb[1]),
        ]
        for idx, (vt, mt) in enumerate(steps):
            nc.tensor.matmul(
                pfin, lhsT=vt, rhs=mt,
                start=(idx == 0), stop=(idx == len(steps) - 1),
            )
        out_sb = T("out_sb", (BPC, C))
        nc.vector.tensor_mul(out=out_sb, in0=pfin, in1=y_row)
        base = outp.ap()
        dst = bass.AP(tensor=base.tensor, offset=0, ap=[[C, BPC], [1, C]])
        nc.sync.dma_start(out=dst, in_=out_sb)

    nc.compile()
    return nc


_CACHE = {}


def _get_nc():
    if "nc" not in _CACHE:
        _CACHE["nc"] = _build()
    return _CACHE["nc"]


def _host_prep(inputs):
    import ml_dtypes

    f32, f16 = np.float32, np.float16
    bf16 = ml_dtypes.bfloat16
    W1 = np.asarray(inputs["W1"], f32)
    W2 = np.asarray(inputs["W2"], f32)
    Ws1 = np.asarray(inputs["Ws1"], f32)
    Ws2 = np.asarray(inputs["Ws2"], f32)
    b1 = np.asarray(inputs["b1"], f32)
    b2 = np.asarray(inputs["b2"], f32)
    bs1 = np.asarray(inputs["bs1"], f32)
    bs2 = np.asarray(inputs["bs2"], f32)
    # center taps of the 3x3 convs; fold the 1/HW mean scale into W1
    w1t = (W1[:, :, 1, 1].T.astype(np.float64) / HW).astype(f32)   # [C, CH]
    w2r = np.concatenate(
        [np.ascontiguousarray(W2[:, :, 1, 1].T), b2.reshape(1, C)], axis=0
    )                                                              # [CH+1, C]
    ws1t = np.ascontiguousarray(Ws1.T)                             # [C, CF]
    ws2t = np.ascontiguousarray(Ws2.T)

    i = np.arange(C, dtype=np.float64)[:, None]
    k = np.arange(CF, dtype=np.float64)[None, :]
    ang = 2.0 * np.pi * i * k / C
    cmat = np.cos(ang).astype(f32)                                 # [C, CF]
    smat = (-np.sin(ang)).astype(f32)

    kk = np.arange(CF, dtype=np.float64)[:, None]
    n = np.arange(C, dtype=np.float64)[None, :]
    ang2 = 2.0 * np.pi * kk * n / C
    alpha = np.full((CF, 1), 2.0)
    alpha[0, 0] = 1.0
    alpha[NYQ, 0] = 1.0
    icrm = (alpha * np.cos(ang2) / C).astype(f32)                  # [CF, C]
    icim = (-alpha * np.sin(ang2) / C).astype(f32)

    wu = np.zeros((128, TOTU), np.uint16)

    def put16(name, arr, dt):  # arr: [rows, cols] fp32
        o = _OFF[name]
        wu[: arr.shape[0], o : o + arr.shape[1]] = (
            arr.astype(dt).view(np.uint16)
        )

    for k3 in range(3):
        put16(f"w1t{k3}", w1t[k3 * 128 : (k3 + 1) * 128, :], bf16)
    put16("w2r", w2r, f16)
    csm = np.concatenate([cmat, smat], axis=1)                     # [C, 2CF]
    w12 = np.concatenate([ws1t, ws2t], axis=1)
    for nm, mat in (("csm", csm), ("w12", w12)):
        for k3 in range(3):
            put16(f"{nm}{k3}", mat[k3 * 128 : (k3 + 1) * 128, :], f16)
    put16("bsrow", np.concatenate([bs1, bs2]).reshape(1, 2 * CF), f16)
    put16("id2h", np.eye(BPC, dtype=f32), f16)
    for j, (s, l) in enumerate(FC):
        put16(f"icr{j}", icrm[s : s + l, :], f16)
        put16(f"ici{j}", icim[s : s + l, :], f16)
    put16("ones2", np.ones((1, BPC), f32), f16)

    wfp = np.zeros((128, TOTF), f32)
    wfp[:CH, 0] = b1
    wfp[:BPC, 1 : 1 + BPC] = np.eye(BPC, dtype=f32)
    return {"wu": wu, "wf": wfp}


def kernel(**inputs):
    x = np.asarray(inputs["x"], np.float32)
    base = _host_prep(inputs)
    nc = _get_nc()
    in_maps = [
        dict(base, xs=np.ascontiguousarray(x[i * BPC : (i + 1) * BPC]))
        for i in range(NCORES)
    ]
    res = run_bass_kernel_spmd(nc, in_maps, list(range(NCORES))).results
    return np.concatenate([res[i]["out"] for i in range(NCORES)], axis=0)


# revision 22
# speedup vs baseline: 1.0034x; 1.0034x over previous
"""FFT-block kernel for Trainium2 (8 NeuronCores, batch-data-parallel).

Computation (per sample):
  y0  = mean(x, (H, W))                      [C]
  h   = relu(y0 @ W1c.T + b1)                [C/6]
  y   = sigmoid(h @ W2c.T + b2)              [C]
  s1  = relu(y @ Ws1.T + bs1)                [CF]
  s2  = relu(y @ Ws2.T + bs2)                [CF]
  yf  = rfft(y); amp=|yf|*s1; pha=angle(yf)*s2
  rec = amp*(cos(pha) + i sin(pha)); xr = irfft(rec, C)
  out = (xr * y)[:, None, None]

Strategy: batch dim (16) sharded 2-per-core. The 400MB stream of x feeds a
free-axis reduction; chunks alternate between the DVE reduce and the ACT
accum_out path.  The stream is paced by the 16 DMA engines (~27 GB/s each,
~430 GB/s aggregate), so the tail after the last byte is what matters:

- ACT's stream work is front-loaded so its activation-table loads (sigmoid
  -> sqrt -> arctan -> sin: arctan and sin share trig_and_small, 3 loads
  total) retire before / underneath other work.
- The last row-tile's chunks cascade down to 256 cols so the final reduce
  is short; per-chunk partials land in per-engine column tiles combined by
  one reduce each.
- The MLP/FFT tail runs batch-major: y is computed as a [2, C] row via a
  ones-row affine trick (bias folded into the matmul), PE-transposed into
  [C, 2] columns for the four projections (fp16 weights, 193 moving cols
  into [2, CF] PSUM tiles - bins on the free axis means ONE trig chain,
  not two), and the irfft is 4 fp16 matmuls producing [2, C] directly so
  the output needs no transposes.  All tail matrices are fp16 (bf16's
  0.4% rounding is too coarse for the AC bins of the DFT); w1t is bf16
  because 1/HW folding makes its values fp16-subnormal.
- angle() uses atan2(y,x) = 2*atan(y/(|z|+x)) with the atan argument
  folded into [0,1]; sin/cos use an exact mod-2pi range reduction (fp32
  round-to-int magic).  Nyquist (Im==0 analytically) special-cased via
  sign(Re); DC is provably Re>0 (sum of sigmoids) so it needs no fix.
"""

import numpy as np
from contextlib import ExitStack

import concourse.bass as bass
import concourse.bacc as bacc
import concourse.tile as tile
from concourse import mybir
from concourse.bass_utils import run_bass_kernel_spmd

B, C, H, W = 16, 384, 128, 128
NCORES = 8
BPC = B // NCORES            # 2 samples per core
CH = C // 6                  # 64
CF = C // 2 + 1              # 193 rfft bins
HW = H * W                   # 16384
FP32 = mybir.dt.float32
FP16 = mybir.dt.float16
BF16 = mybir.dt.bfloat16
U16 = mybir.dt.uint16
AF = mybir.ActivationFunctionType
AX = mybir.AxisListType
OP = mybir.AluOpType

F_CHUNK = 4096               # free-dim chunk of the x stream
STREAM_BUFS = 10

KC = [0, 128, 256]           # channel chunks (3 x 128)
FC = [(0, 128), (128, 65)]   # freq-bin chunks (128 + 65)
NYQ = 192                    # Nyquist bin index
MAGIC = 12582912.0           # 1.5 * 2**23: x+MAGIC-MAGIC == round(x) in fp32

# stream chunk schedule: (sizes, engines) per row-tile. 'D' = DVE reduce,
# 'A' = ACT accum.  Strict D/A alternation on the 4096-col chunks: runs of
# consecutive same-engine full chunks stall SBUF ring recycling and the
# DMA engines lose ~15% per-packet throughput for the whole endgame.  The
# cascade keeps the final reduce short; ACT's last chunk (t5/j5) lands
# ~0.6us before the stream ends, leaving room for the sigmoid table load
# in front of y's matmul.
_FULL = [F_CHUNK] * 4
_SCHED = [
    (_FULL, "DADA"),
    (_FULL, "DADA"),
    (_FULL, "DADA"),
    (_FULL, "DADA"),
    (_FULL, "DADA"),
    ([4096, 4096, 4096, 2048, 1024, 512, 256, 256], "DAADADDD"),
]
assert all(sum(s) == HW for s, _ in _SCHED)

# ---- packed-constant column layout (u16 tensor; regions bitcast to
# bf16/fp16 on device) ----
_OFF = {}
_tot = 0


def _alloc_cols(name, ncols):
    global _tot
    _OFF[name] = _tot
    _tot += ncols


for _k in range(3):
    _alloc_cols(f"w1t{_k}", CH)          # bf16 [128, 64]
_alloc_cols("w2r", C)                    # fp16 [65, 384] (row 64 = b2)
for _nm in ("csm", "w12"):               # packed pairs [cm|sm], [ws1|ws2]
    for _k in range(3):
        _alloc_cols(f"{_nm}{_k}", 2 * CF)  # fp16 [128, 386]
_alloc_cols("bsrow", 2 * CF)             # fp16 [1, 386] = [bs1|bs2]
_alloc_cols("id2h", BPC)                 # fp16 eye(2)
for _j in range(2):
    _alloc_cols(f"icr{_j}", C)           # fp16 [128/65, 384]
    _alloc_cols(f"ici{_j}", C)
_alloc_cols("ones2", BPC)                # fp16 [1, 2]
TOTU = _tot
TOTF = 3                                 # fp32: b1 [64,1], id2 [2,2]


def _build():
    nc = bacc.Bacc(
        "TRN2",
        target_bir_lowering=False,
        debug=False,
        enable_asserts=False,
        num_devices=NCORES,
    )

    xs = nc.dram_tensor("xs", [BPC, C, H, W], FP32, kind="ExternalInput")
    wu = nc.dram_tensor("wu", [128, TOTU], U16, kind="ExternalInput")
    wf = nc.dram_tensor("wf", [128, TOTF], FP32, kind="ExternalInput")
    outp = nc.dram_tensor("out", [BPC, C, 1, 1], FP32, kind="ExternalOutput")

    with tile.TileContext(nc) as tc, ExitStack() as ctx:
        persist = ctx.enter_context(tc.tile_pool(name="persist", bufs=1))
        stream = ctx.enter_context(tc.tile_pool(name="stream", bufs=STREAM_BUFS))
        ps_mm = ctx.enter_context(
            tc.tile_pool(name="ps_mm", bufs=6, space=bass.MemorySpace.PSUM)
        )
        ps_fin = ctx.enter_context(
            tc.tile_pool(name="ps_fin", bufs=1, space=bass.MemorySpace.PSUM)
        )

        # ---- constants in two DMAs on the ACT queue (sync queue runs the
        # x stream exclusively) ----
        wub = persist.tile([128, TOTU], U16, tag="wub", name="wub")
        nc.scalar.dma_start(out=wub, in_=wu[:, :])
        wfb = persist.tile([128, TOTF], FP32, tag="wfb", name="wfb")
        nc.scalar.dma_start(out=wfb, in_=wf[:, :])

        def cs16(name, rows, ncols, dt):
            o = _OFF[name]
            return wub[:rows, o : o + ncols].bitcast(dt)

        w1t_sb = [cs16(f"w1t{k}", 128, CH, BF16) for k in range(3)]
        w2r_sb = cs16("w2r", CH + 1, C, FP16)
        mats = {
            nm: [cs16(f"{nm}{k}", 128, 2 * CF, FP16) for k in range(3)]
            for nm in ("csm", "w12")
        }
        bsrow = cs16("bsrow", 1, 2 * CF, FP16)
        id2h = cs16("id2h", BPC, BPC, FP16)
        icr_sb = [cs16(f"icr{j}", l, C, FP16) for j, (s, l) in enumerate(FC)]
        ici_sb = [cs16(f"ici{j}", l, C, FP16) for j, (s, l) in enumerate(FC)]
        ones2 = cs16("ones2", 1, BPC, FP16)
        b1_sb = wfb[:CH, 0:1]
        id2 = wfb[:BPC, 1 : 1 + BPC]

        def T(tag, shape=(BPC, CF), dt=FP32):
            return persist.tile(list(shape), dt, tag=tag, name=tag)

        # h2 = [h; ones] (fp16) - ones row set once, rows 0-63 written later
        h2 = T("h2", (CH + 1, BPC), FP16)
        nc.vector.memset(h2[CH : CH + 1, :], 1.0)

        # ---- phase 1: stream x, per-(b,c)-row sums over the spatial axis.
        # DVE partials land in columns of Dpart_t, ACT accums in Apart_t;
        # one reduce each + one add finalizes yt as bf16 (matmul rhs).
        xrows = xs.rearrange("b c h w -> (b c) (h w)")
        dummy = persist.tile([128, F_CHUNK], BF16, tag="dummy", name="dummy")
        ytb = [T(f"ytb{k}", (128, BPC), BF16) for k in range(3)]
        ph = [None, None]

        for t, (sizes, engines) in enumerate(_SCHED):
            b, k = divmod(t, 3)
            nD = engines.count("D")
            nA = engines.count("A")
            Dpart = T(f"Dp{t}", (128, nD))
            Apart = T(f"Ap{t}", (128, nA))
            off = jD = jA = 0
            for j, fch in enumerate(sizes):
                chk = stream.tile([128, fch], FP32, tag="stream", name=f"chk{t}_{j}")
                nc.sync.dma_start(
                    out=chk,
                    in_=xrows[t * 128 : (t + 1) * 128, off : off + fch],
                )
                off += fch
                if engines[j] == "D":
                    nc.vector.reduce_sum(out=Dpart[:, jD : jD + 1], in_=chk, axis=AX.X)
                    jD += 1
                else:
                    nc.scalar.activation(
                        out=dummy[:, :fch], in_=chk, func=AF.Identity,
                        accum_out=Apart[:, jA : jA + 1],
                    )
                    jA += 1
            Dsum = T(f"Ds{t}", (128, 1))
            nc.vector.reduce_sum(out=Dsum, in_=Dpart, axis=AX.X)
            Asum = T(f"As{t}", (128, 1))
            nc.vector.reduce_sum(out=Asum, in_=Apart, axis=AX.X)
            nc.vector.tensor_add(out=ytb[k][:, b : b + 1], in0=Dsum, in1=Asum)

            # squeeze matmul contribution for this (b, k) right away; only
            # the (b=1, k=2) one lands after the stream.
            if k == 0:
                ph[b] = ps_mm.tile([CH, 1], FP32, tag="mm", name=f"ph{b}")
            nc.tensor.matmul(
                ph[b], lhsT=w1t_sb[k], rhs=ytb[k][:, b : b + 1],
                start=(k == 0), stop=(k == 2),
            )
            if k == 2:  # h = relu(ph + b1) -> fp16 column of h2
                htmp = T(f"htmp{b}", (CH, 1))
                nc.vector.tensor_add(out=htmp, in0=ph[b], in1=b1_sb)
                nc.vector.tensor_scalar_max(
                    out=h2[:CH, b : b + 1], in0=htmp, scalar1=0.0
                )

        # ---- phase 2: y as a [2, C] row (bias via the ones row) ----
        py = ps_mm.tile([BPC, C], FP32, tag="mm", name="py")
        nc.tensor.matmul(py, lhsT=h2, rhs=w2r_sb, start=True, stop=True)
        y_row = T("y_row", (BPC, C))
        nc.scalar.activation(out=y_row, in_=py, func=AF.Sigmoid)
        # dummy sqrt on ready data: pulls the sqrt-set table load to right
        # after the sigmoid instead of behind the (projection-gated) Square
        scr = T("scr", (1, BPC))
        nc.scalar.activation(out=scr, in_=h2[CH : CH + 1, :], func=AF.Sqrt)

        # y columns for the projections: PE-transpose 128-col slices
        y_col = []
        for m, s in enumerate(KC):
            yT = ps_mm.tile([128, BPC], FP32, tag="mm", name=f"yT{m}")
            nc.tensor.transpose(yT, y_row[:, s : s + 128], id2)
            yc = T(f"yc{m}", (128, BPC), FP16)
            nc.vector.tensor_copy(out=yc, in_=yT)
            y_col.append(yc)

        # ---- projections, pairwise packed: [2, 386] PSUM tiles hold
        # [re|im] and [s1|s2] (bins on the free axis).  re/im first (they
        # gate the trig chain); s1/s2 bias via a rank-1 ones matmul.
        prem = ps_mm.tile([BPC, 2 * CF], FP32, tag="mm", name="prem")
        for k in range(3):
            nc.tensor.matmul(prem, lhsT=y_col[k], rhs=mats["csm"][k],
                             start=(k == 0), stop=(k == 2))
        ps12 = ps_mm.tile([BPC, 2 * CF], FP32, tag="mm", name="ps12")
        for k in range(3):
            nc.tensor.matmul(ps12, lhsT=y_col[k], rhs=mats["w12"][k],
                             start=(k == 0), stop=False)
        nc.tensor.matmul(ps12, lhsT=ones2, rhs=bsrow, start=False, stop=True)
        pre = prem[:, :CF]
        pim = prem[:, CF:]
        ps1 = ps12[:, :CF]
        ps2 = ps12[:, CF:]

        # ---- trig chain on [2, 193] ----
        # One serial DVE chain; off-chain ops are queued into the windows
        # where DVE would otherwise wait on an ACT result (sqrt/arctan).
        # atan2 via one approximate reciprocal: u = min(|im|, |z|+re) /
        # max(|im|, |z|+re); the |t|>1 fold becomes (|im| > |z|+re) and the
        # sign/fold application collapses to at = a*g1 + g0.
        crem = T("crem", (BPC, 2 * CF))  # one wide PSUM->SBUF copy
        nc.vector.tensor_copy(out=crem, in_=prem)
        cre = crem[:, :CF]
        cim = crem[:, CF:]
        r2 = T("r2")
        nc.vector.tensor_mul(out=r2, in0=cre, in1=pre)
        i2 = T("i2")
        nc.vector.tensor_mul(out=i2, in0=cim, in1=pim)
        nc.vector.tensor_add(out=r2, in0=r2, in1=i2)
        amp0 = T("amp0")
        nc.scalar.activation(out=amp0, in_=r2, func=AF.Sqrt)
        # fillers while ACT runs sqrt:
        absim = T("absim")  # |im| = max(-im, im)
        nc.vector.scalar_tensor_tensor(
            out=absim, in0=cim, scalar=-1.0, in1=cim, op0=OP.mult, op1=OP.max
        )
        sgn = T("sgn")  # 2*(im>0) - 1
        nc.vector.tensor_scalar(
            out=sgn, in0=cim, scalar1=0.0, scalar2=2.0, op0=OP.is_gt, op1=OP.mult
        )
        nc.vector.tensor_scalar_sub(out=sgn, in0=sgn, scalar1=1.0)
        fpn = T("fpn", (BPC, 1))  # Nyquist: Re>0 (Im==0 analytically there)
        nc.vector.tensor_scalar(
            out=fpn, in0=pre[:, NYQ : NYQ + 1], scalar1=0.0, scalar2=None,
            op0=OP.is_gt,
        )
        den0 = T("den0")  # |z| + re >= 0
        nc.vector.tensor_add(out=den0, in0=amp0, in1=pre)
        mx = T("mx")
        nc.vector.tensor_tensor(out=mx, in0=absim, in1=den0, op=OP.max)
        rmx = T("rmx")  # 1/mx, fast approx (~18 bits)
        nc.vector.reciprocal_approx_fast(out=rmx, in_=mx)
        mn = T("mn")
        nc.vector.tensor_tensor(out=mn, in0=absim, in1=den0, op=OP.min)
        u = T("u")
        nc.vector.tensor_mul(out=u, in0=mn, in1=rmx)
        a = T("a")  # atan(u) in [0, pi/4]
        nc.scalar.activation(out=a, in_=u, func=AF.Arctan)
        # fillers while ACT runs arctan: fold masks + s1/s2 post-ops
        fgt = T("fgt")  # |tan(angle/2)| > 1
        nc.vector.tensor_tensor(out=fgt, in0=absim, in1=den0, op=OP.is_gt)
        g1 = T("g1")  # sgn*(1-2*fgt)
        nc.vector.tensor_scalar(
            out=g1, in0=fgt, scalar1=-2.0, scalar2=1.0, op0=OP.mult, op1=OP.add
        )
        nc.vector.tensor_mul(out=g1, in0=g1, in1=sgn)
        g0 = T("g0")  # sgn*(pi/2)*fgt
        nc.vector.scalar_tensor_tensor(
            out=g0, in0=sgn, scalar=float(np.pi / 2), in1=fgt,
            op0=OP.mult, op1=OP.mult,
        )
        s1r = T("s1r")  # relu(ps1)
        nc.vector.tensor_scalar_max(out=s1r, in0=ps1, scalar1=0.0)
        s2s = T("s2s")  # relu(ps2)/pi
        nc.vector.tensor_scalar(
            out=s2s, in0=ps2, scalar1=0.0, scalar2=float(1.0 / np.pi),
            op0=OP.max, op1=OP.mult,
        )
        amp = T("amp")  # |z|*s1
        nc.vector.tensor_mul(out=amp, in0=amp0, in1=s1r)
        at = T("at")  # angle/2 (signed) = a*g1 + g0
        nc.vector.tensor_mul(out=at, in0=a, in1=g1)
        nc.vector.tensor_add(out=at, in0=at, in1=g0)
        # Nyquist: angle is exactly 0 (Re>0) or pi: at = pi/2 * (1 - (Re>0))
        nc.vector.tensor_scalar(
            out=at[:, NYQ : NYQ + 1], in0=fpn,
            scalar1=float(-np.pi / 2), scalar2=float(np.pi / 2),
            op0=OP.mult, op1=OP.add,
        )
        r_ = T("r_")  # pha / 2pi
        nc.vector.tensor_mul(out=r_, in0=at, in1=s2s)
        # sin branch first (irfft consumes ri first), cos follows
        n1 = T("n1")
        nc.vector.tensor_scalar(
            out=n1, in0=r_, scalar1=MAGIC, scalar2=MAGIC, op0=OP.add, op1=OP.subtract
        )
        nc.vector.tensor_sub(out=n1, in0=r_, in1=n1)
        sn = T("sn")
        nc.scalar.activation(out=sn, in_=n1, func=AF.Sin, scale=float(2 * np.pi))
        # cos arg from the sin arg: frac2 = (frac1 + 0.25) - (frac1 > 0.25)
        # stays in [-0.5, 0.5] (fillers while ACT runs the first sin)
        q4 = T("q4")
        nc.vector.tensor_scalar(
            out=q4, in0=n1, scalar1=0.25, scalar2=None, op0=OP.is_gt
        )
        n2 = T("n2")
        nc.vector.scalar_tensor_tensor(
            out=n2, in0=n1, scalar=0.25, in1=q4, op0=OP.add, op1=OP.subtract
        )
        cs = T("cs")
        nc.scalar.activation(out=cs, in_=n2, func=AF.Sin, scale=float(2 * np.pi))
        ri = T("ri", dt=FP16)
        nc.vector.tensor_mul(out=ri, in0=amp, in1=sn)
        rr = T("rr", dt=FP16)
        nc.vector.tensor_mul(out=rr, in0=amp, in1=cs)

        # ---- irfft as 4 fp16 matmuls into [2, C]; rec transposed into
        # [bins, 2] fp16 columns first ----
        recb = {}
        for nm, src in (("ri", ri), ("rr", rr)):
            for j, (s, l) in enumerate(FC):
                rT = ps_mm.tile([l, BPC], FP16, tag="mm", name=f"{nm}T{j}")
                nc.tensor.transpose(rT, src[:, s : s + l], id2h)
                rb = T(f"{nm}b{j}", (l, BPC), FP16)
                nc.vector.tensor_copy(out=rb, in_=rT)
                recb[(nm, j)] = rb
        pfin = ps_fin.tile([BPC, C], FP32, tag="pfin", name="pfin")
        steps = [
            (recb[("ri", 0)], ici_sb[0]), (recb[("rr", 0)], icr_sb[0]),
            (recb[("ri", 1)], ici_sb[1]), (recb[("rr", 1)], icr_sb[1]),
        ]
        for idx, (vt, mt) in enumerate(steps):
            nc.tensor.matmul(
                pfin, lhsT=vt, rhs=mt,
                start=(idx == 0), stop=(idx == len(steps) - 1),
            )
        out_sb = T("out_sb", (BPC, C))
        nc.vector.tensor_mul(out=out_sb, in0=pfin, in1=y_row)
        base = outp.ap()
        dst = bass.AP(tensor=base.tensor, offset=0, ap=[[C, BPC], [1, C]])
        nc.sync.dma_start(out=dst, in_=out_sb)

    nc.compile()
    return nc


_CACHE = {}


def _get_nc():
    if "nc" not in _CACHE:
        _CACHE["nc"] = _build()
    return _CACHE["nc"]


def _host_prep(inputs):
    import ml_dtypes

    f32, f16 = np.float32, np.float16
    bf16 = ml_dtypes.bfloat16
    W1 = np.asarray(inputs["W1"], f32)
    W2 = np.asarray(inputs["W2"], f32)
    Ws1 = np.asarray(inputs["Ws1"], f32)
    Ws2 = np.asarray(inputs["Ws2"], f32)
    b1 = np.asarray(inputs["b1"], f32)
    b2 = np.asarray(inputs["b2"], f32)
    bs1 = np.asarray(inputs["bs1"], f32)
    bs2 = np.asarray(inputs["bs2"], f32)
    # center taps of the 3x3 convs; fold the 1/HW mean scale into W1
    w1t = (W1[:, :, 1, 1].T.astype(np.float64) / HW).astype(f32)   # [C, CH]
    w2r = np.concatenate(
        [np.ascontiguousarray(W2[:, :, 1, 1].T), b2.reshape(1, C)], axis=0
    )                                                              # [CH+1, C]
    ws1t = np.ascontiguousarray(Ws1.T)                             # [C, CF]
    ws2t = np.ascontiguousarray(Ws2.T)

    i = np.arange(C, dtype=np.float64)[:, None]
    k = np.arange(CF, dtype=np.float64)[None, :]
    ang = 2.0 * np.pi * i * k / C
    cmat = np.cos(ang).astype(f32)                                 # [C, CF]
    smat = (-np.sin(ang)).astype(f32)

    kk = np.arange(CF, dtype=np.float64)[:, None]
    n = np.arange(C, dtype=np.float64)[None, :]
    ang2 = 2.0 * np.pi * kk * n / C
    alpha = np.full((CF, 1), 2.0)
    alpha[0, 0] = 1.0
    alpha[NYQ, 0] = 1.0
    icrm = (alpha * np.cos(ang2) / C).astype(f32)                  # [CF, C]
    icim = (-alpha * np.sin(ang2) / C).astype(f32)

    wu = np.zeros((128, TOTU), np.uint16)

    def put16(name, arr, dt):  # arr: [rows, cols] fp32
        o = _OFF[name]
        wu[: arr.shape[0], o : o + arr.shape[1]] = (
            arr.astype(dt).view(np.uint16)
        )

    for k3 in range(3):
        put16(f"w1t{k3}", w1t[k3 * 128 : (k3 + 1) * 128, :], bf16)
    put16("w2r", w2r, f16)
    csm = np.concatenate([cmat, smat], axis=1)                     # [C, 2CF]
    w12 = np.concatenate([ws1t, ws2t], axis=1)
    for nm, mat in (("csm", csm), ("w12", w12)):
        for k3 in range(3):
            put16(f"{nm}{k3}", mat[k3 * 128 : (k3 + 1) * 128, :], f16)
    put16("bsrow", np.concatenate([bs1, bs2]).reshape(1, 2 * CF), f16)
    put16("id2h", np.eye(BPC, dtype=f32), f16)
    for j, (s, l) in enumerate(FC):
        put16(f"icr{j}", icrm[s : s + l, :], f16)
        put16(f"ici{j}", icim[s : s + l, :], f16)
    put16("ones2", np.ones((1, BPC), f32), f16)

    wfp = np.zeros((128, TOTF), f32)
    wfp[:CH, 0] = b1
    wfp[:BPC, 1 : 1 + BPC] = np.eye(BPC, dtype=f32)
    return {"wu": wu, "wf": wfp}


def kernel(**inputs):
    x = np.asarray(inputs["x"], np.float32)
    base = _host_prep(inputs)
    nc = _get_nc()
    in_maps = [
        dict(base, xs=np.ascontiguousarray(x[i * BPC : (i + 1) * BPC]))
        for i in range(NCORES)
    ]
    res = run_bass_kernel_spmd(nc, in_maps, list(range(NCORES))).results
    return np.concatenate([res[i]["out"] for i in range(NCORES)], axis=0)


# revision 23
# speedup vs baseline: 1.0058x; 1.0023x over previous
"""FFT-block kernel for Trainium2 (8 NeuronCores, batch-data-parallel).

Computation (per sample):
  y0  = mean(x, (H, W))                      [C]
  h   = relu(y0 @ W1c.T + b1)                [C/6]
  y   = sigmoid(h @ W2c.T + b2)              [C]
  s1  = relu(y @ Ws1.T + bs1)                [CF]
  s2  = relu(y @ Ws2.T + bs2)                [CF]
  yf  = rfft(y); amp=|yf|*s1; pha=angle(yf)*s2
  rec = amp*(cos(pha) + i sin(pha)); xr = irfft(rec, C)
  out = (xr * y)[:, None, None]

Strategy: batch dim (16) sharded 2-per-core. The 400MB stream of x feeds a
free-axis reduction; chunks alternate between the DVE reduce and the ACT
accum_out path.  The stream is paced by the 16 DMA engines (~27 GB/s each,
~430 GB/s aggregate), so the tail after the last byte is what matters:

- ACT's stream work is front-loaded so its activation-table loads (sigmoid
  -> sqrt -> arctan -> sin: arctan and sin share trig_and_small, 3 loads
  total) retire before / underneath other work.
- The last row-tile's chunks cascade down to 256 cols so the final reduce
  is short; per-chunk partials land in per-engine column tiles combined by
  one reduce each.
- The MLP/FFT tail runs batch-major: y is computed as a [2, C] row via a
  ones-row affine trick (bias folded into the matmul), PE-transposed into
  [C, 2] columns for the four projections (fp16 weights, 193 moving cols
  into [2, CF] PSUM tiles - bins on the free axis means ONE trig chain,
  not two), and the irfft is 4 fp16 matmuls producing [2, C] directly so
  the output needs no transposes.  All tail matrices are fp16 (bf16's
  0.4% rounding is too coarse for the AC bins of the DFT); w1t is bf16
  because 1/HW folding makes its values fp16-subnormal.
- angle() uses atan2(y,x) = 2*atan(y/(|z|+x)) with the atan argument
  folded into [0,1]; sin/cos use an exact mod-2pi range reduction (fp32
  round-to-int magic).  Nyquist (Im==0 analytically) special-cased via
  sign(Re); DC is provably Re>0 (sum of sigmoids) so it needs no fix.
"""

import numpy as np
from contextlib import ExitStack

import concourse.bass as bass
import concourse.bacc as bacc
import concourse.tile as tile
from concourse import mybir
from concourse.bass_utils import run_bass_kernel_spmd

B, C, H, W = 16, 384, 128, 128
NCORES = 8
BPC = B // NCORES            # 2 samples per core
CH = C // 6                  # 64
CF = C // 2 + 1              # 193 rfft bins
HW = H * W                   # 16384
FP32 = mybir.dt.float32
FP16 = mybir.dt.float16
BF16 = mybir.dt.bfloat16
U16 = mybir.dt.uint16
AF = mybir.ActivationFunctionType
AX = mybir.AxisListType
OP = mybir.AluOpType

F_CHUNK = 4096               # free-dim chunk of the x stream
STREAM_BUFS = 10

KC = [0, 128, 256]           # channel chunks (3 x 128)
FC = [(0, 128), (128, 65)]   # freq-bin chunks (128 + 65)
NYQ = 192                    # Nyquist bin index
MAGIC = 12582912.0           # 1.5 * 2**23: x+MAGIC-MAGIC == round(x) in fp32

# stream chunk schedule: (sizes, engines) per row-tile. 'D' = DVE reduce,
# 'A' = ACT accum.  Strict D/A alternation on the 4096-col chunks: runs of
# consecutive same-engine full chunks stall SBUF ring recycling and the
# DMA engines lose ~15% per-packet throughput for the whole endgame.  The
# cascade keeps the final reduce short; ACT's last chunk (t5/j5) lands
# ~0.6us before the stream ends, leaving room for the sigmoid table load
# in front of y's matmul.
_FULL = [F_CHUNK] * 4
_SCHED = [
    (_FULL, "DADA"),
    (_FULL, "DADA"),
    (_FULL, "DADA"),
    (_FULL, "DADA"),
    (_FULL, "DADA"),
    ([4096, 4096, 4096, 2048, 1024, 512, 256, 256], "DAADADDD"),
]
assert all(sum(s) == HW for s, _ in _SCHED)

# ---- packed-constant column layout (u16 tensor; regions bitcast to
# bf16/fp16 on device) ----
_OFF = {}
_tot = 0


def _alloc_cols(name, ncols):
    global _tot
    _OFF[name] = _tot
    _tot += ncols


for _k in range(3):
    _alloc_cols(f"w1t{_k}", CH)          # bf16 [128, 64]
_alloc_cols("w2r", C)                    # fp16 [65, 384] (row 64 = b2)
for _nm in ("csm", "w12"):               # packed pairs [cm|sm], [ws1|ws2]
    for _k in range(3):
        _alloc_cols(f"{_nm}{_k}", 2 * CF)  # fp16 [128, 386]
_alloc_cols("bsrow", 2 * CF)             # fp16 [1, 386] = [bs1|bs2]
_alloc_cols("id2h", BPC)                 # fp16 eye(2)
for _j in range(2):
    _alloc_cols(f"icr{_j}", C)           # fp16 [128/65, 384]
    _alloc_cols(f"ici{_j}", C)
_alloc_cols("ones2", BPC)                # fp16 [1, 2]
TOTU = _tot
TOTF = 3                                 # fp32: b1 [64,1], id2 [2,2]


def _build():
    nc = bacc.Bacc(
        "TRN2",
        target_bir_lowering=False,
        debug=False,
        enable_asserts=False,
        num_devices=NCORES,
    )

    xs = nc.dram_tensor("xs", [BPC, C, H, W], FP32, kind="ExternalInput")
    wu = nc.dram_tensor("wu", [128, TOTU], U16, kind="ExternalInput")
    wf = nc.dram_tensor("wf", [128, TOTF], FP32, kind="ExternalInput")
    outp = nc.dram_tensor("out", [BPC, C, 1, 1], FP32, kind="ExternalOutput")

    with tile.TileContext(nc) as tc, ExitStack() as ctx:
        persist = ctx.enter_context(tc.tile_pool(name="persist", bufs=1))
        stream = ctx.enter_context(tc.tile_pool(name="stream", bufs=STREAM_BUFS))
        ps_mm = ctx.enter_context(
            tc.tile_pool(name="ps_mm", bufs=6, space=bass.MemorySpace.PSUM)
        )
        ps_fin = ctx.enter_context(
            tc.tile_pool(name="ps_fin", bufs=1, space=bass.MemorySpace.PSUM)
        )

        # ---- constants in two DMAs on the ACT queue (sync queue runs the
        # x stream exclusively) ----
        wub = persist.tile([128, TOTU], U16, tag="wub", name="wub")
        nc.scalar.dma_start(out=wub, in_=wu[:, :])
        wfb = persist.tile([128, TOTF], FP32, tag="wfb", name="wfb")
        nc.scalar.dma_start(out=wfb, in_=wf[:, :])

        def cs16(name, rows, ncols, dt):
            o = _OFF[name]
            return wub[:rows, o : o + ncols].bitcast(dt)

        w1t_sb = [cs16(f"w1t{k}", 128, CH, BF16) for k in range(3)]
        w2r_sb = cs16("w2r", CH + 1, C, FP16)
        mats = {
            nm: [cs16(f"{nm}{k}", 128, 2 * CF, FP16) for k in range(3)]
            for nm in ("csm", "w12")
        }
        bsrow = cs16("bsrow", 1, 2 * CF, FP16)
        id2h = cs16("id2h", BPC, BPC, FP16)
        icr_sb = [cs16(f"icr{j}", l, C, FP16) for j, (s, l) in enumerate(FC)]
        ici_sb = [cs16(f"ici{j}", l, C, FP16) for j, (s, l) in enumerate(FC)]
        ones2 = cs16("ones2", 1, BPC, FP16)
        b1_sb = wfb[:CH, 0:1]
        id2 = wfb[:BPC, 1 : 1 + BPC]

        def T(tag, shape=(BPC, CF), dt=FP32):
            return persist.tile(list(shape), dt, tag=tag, name=tag)

        # h2 = [h; ones] (fp16) - ones row set once, rows 0-63 written later
        h2 = T("h2", (CH + 1, BPC), FP16)
        nc.vector.memset(h2[CH : CH + 1, :], 1.0)

        # ---- phase 1: stream x, per-(b,c)-row sums over the spatial axis.
        # DVE partials land in columns of Dpart_t, ACT accums in Apart_t;
        # one reduce each + one add finalizes yt as bf16 (matmul rhs).
        xrows = xs.rearrange("b c h w -> (b c) (h w)")
        dummy = persist.tile([128, F_CHUNK], BF16, tag="dummy", name="dummy")
        ytb = [T(f"ytb{k}", (128, BPC), BF16) for k in range(3)]
        ph = [None, None]

        for t, (sizes, engines) in enumerate(_SCHED):
            b, k = divmod(t, 3)
            nD = engines.count("D")
            nA = engines.count("A")
            Dpart = T(f"Dp{t}", (128, nD))
            Apart = T(f"Ap{t}", (128, nA))
            off = jD = jA = 0
            for j, fch in enumerate(sizes):
                chk = stream.tile([128, fch], FP32, tag="stream", name=f"chk{t}_{j}")
                nc.sync.dma_start(
                    out=chk,
                    in_=xrows[t * 128 : (t + 1) * 128, off : off + fch],
                )
                off += fch
                if engines[j] == "D":
                    nc.vector.reduce_sum(out=Dpart[:, jD : jD + 1], in_=chk, axis=AX.X)
                    jD += 1
                else:
                    nc.scalar.activation(
                        out=dummy[:, :fch], in_=chk, func=AF.Identity,
                        accum_out=Apart[:, jA : jA + 1],
                    )
                    jA += 1
            Dsum = T(f"Ds{t}", (128, 1))
            nc.vector.reduce_sum(out=Dsum, in_=Dpart, axis=AX.X)
            Asum = T(f"As{t}", (128, 1))
            nc.vector.reduce_sum(out=Asum, in_=Apart, axis=AX.X)
            nc.vector.tensor_add(out=ytb[k][:, b : b + 1], in0=Dsum, in1=Asum)

            # squeeze matmul contribution for this (b, k) right away; only
            # the (b=1, k=2) one lands after the stream.
            if k == 0:
                ph[b] = ps_mm.tile([CH, 1], FP32, tag="mm", name=f"ph{b}")
            nc.tensor.matmul(
                ph[b], lhsT=w1t_sb[k], rhs=ytb[k][:, b : b + 1],
                start=(k == 0), stop=(k == 2),
            )
            if k == 2:  # h = relu(ph + b1) -> fp16 column of h2
                htmp = T(f"htmp{b}", (CH, 1))
                nc.vector.tensor_add(out=htmp, in0=ph[b], in1=b1_sb)
                nc.vector.tensor_scalar_max(
                    out=h2[:CH, b : b + 1], in0=htmp, scalar1=0.0
                )

        # ---- phase 2: y as a [2, C] row (bias via the ones row) ----
        py = ps_mm.tile([BPC, C], FP32, tag="mm", name="py")
        nc.tensor.matmul(py, lhsT=h2, rhs=w2r_sb, start=True, stop=True)
        y_row = T("y_row", (BPC, C))
        nc.scalar.activation(out=y_row, in_=py, func=AF.Sigmoid)
        # dummy sqrt on ready data: pulls the sqrt-set table load to right
        # after the sigmoid instead of behind the (projection-gated) Square
        scr = T("scr", (1, BPC))
        nc.scalar.activation(out=scr, in_=h2[CH : CH + 1, :], func=AF.Sqrt)

        # y columns for the projections: PE-transpose 128-col slices
        y_col = []
        for m, s in enumerate(KC):
            yT = ps_mm.tile([128, BPC], FP32, tag="mm", name=f"yT{m}")
            nc.tensor.transpose(yT, y_row[:, s : s + 128], id2)
            yc = T(f"yc{m}", (128, BPC), FP16)
            nc.vector.tensor_copy(out=yc, in_=yT)
            y_col.append(yc)

        # ---- projections, pairwise packed: [2, 386] PSUM tiles hold
        # [re|im] and [s1|s2] (bins on the free axis).  re/im first (they
        # gate the trig chain); s1/s2 bias via a rank-1 ones matmul.
        prem = ps_mm.tile([BPC, 2 * CF], FP32, tag="mm", name="prem")
        for k in range(3):
            nc.tensor.matmul(prem, lhsT=y_col[k], rhs=mats["csm"][k],
                             start=(k == 0), stop=(k == 2))
        ps12 = ps_mm.tile([BPC, 2 * CF], FP32, tag="mm", name="ps12")
        for k in range(3):
            nc.tensor.matmul(ps12, lhsT=y_col[k], rhs=mats["w12"][k],
                             start=(k == 0), stop=False)
        nc.tensor.matmul(ps12, lhsT=ones2, rhs=bsrow, start=False, stop=True)
        pre = prem[:, :CF]
        pim = prem[:, CF:]
        ps1 = ps12[:, :CF]
        ps2 = ps12[:, CF:]

        # ---- trig chain on [2, 193] ----
        # One serial DVE chain; off-chain ops are queued into the windows
        # where DVE would otherwise wait on an ACT result (sqrt/arctan).
        # atan2 via one approximate reciprocal: u = min(|im|, |z|+re) /
        # max(|im|, |z|+re); the |t|>1 fold becomes (|im| > |z|+re) and the
        # sign/fold application collapses to at = a*g1 + g0.
        # re^2+im^2 via one ACT Square on the packed tile (Square lives in
        # every table set - no load) + one DVE add; only im needs an SBUF
        # copy (for the mask ops), re is read straight from PSUM.
        sq = T("sq", (BPC, 2 * CF))
        nc.scalar.activation(out=sq, in_=prem, func=AF.Square)
        cim = T("cim")
        nc.vector.tensor_copy(out=cim, in_=pim)
        r2 = T("r2")
        nc.vector.tensor_add(out=r2, in0=sq[:, :CF], in1=sq[:, CF:])
        amp0 = T("amp0")
        nc.scalar.activation(out=amp0, in_=r2, func=AF.Sqrt)
        # fillers while ACT runs sqrt:
        absim = T("absim")  # |im| = max(-im, im)
        nc.vector.scalar_tensor_tensor(
            out=absim, in0=cim, scalar=-1.0, in1=cim, op0=OP.mult, op1=OP.max
        )
        sgn = T("sgn")  # 2*(im>0) - 1
        nc.vector.tensor_scalar(
            out=sgn, in0=cim, scalar1=0.0, scalar2=2.0, op0=OP.is_gt, op1=OP.mult
        )
        nc.vector.tensor_scalar_sub(out=sgn, in0=sgn, scalar1=1.0)
        fpn = T("fpn", (BPC, 1))  # Nyquist: Re>0 (Im==0 analytically there)
        nc.vector.tensor_scalar(
            out=fpn, in0=pre[:, NYQ : NYQ + 1], scalar1=0.0, scalar2=None,
            op0=OP.is_gt,
        )
        den0 = T("den0")  # |z| + re >= 0
        nc.vector.tensor_add(out=den0, in0=amp0, in1=pre)
        mx = T("mx")
        nc.vector.tensor_tensor(out=mx, in0=absim, in1=den0, op=OP.max)
        rmx = T("rmx")  # 1/mx, fast approx (~18 bits)
        nc.vector.reciprocal_approx_fast(out=rmx, in_=mx)
        mn = T("mn")
        nc.vector.tensor_tensor(out=mn, in0=absim, in1=den0, op=OP.min)
        u = T("u")
        nc.vector.tensor_mul(out=u, in0=mn, in1=rmx)
        a = T("a")  # atan(u) in [0, pi/4]
        nc.scalar.activation(out=a, in_=u, func=AF.Arctan)
        # fillers while ACT runs arctan: fold masks + s1/s2 post-ops
        fgt = T("fgt")  # |tan(angle/2)| > 1
        nc.vector.tensor_tensor(out=fgt, in0=absim, in1=den0, op=OP.is_gt)
        g1 = T("g1")  # sgn*(1-2*fgt)
        nc.vector.tensor_scalar(
            out=g1, in0=fgt, scalar1=-2.0, scalar2=1.0, op0=OP.mult, op1=OP.add
        )
        nc.vector.tensor_mul(out=g1, in0=g1, in1=sgn)
        g0 = T("g0")  # sgn*(pi/2)*fgt
        nc.vector.scalar_tensor_tensor(
            out=g0, in0=sgn, scalar=float(np.pi / 2), in1=fgt,
            op0=OP.mult, op1=OP.mult,
        )
        s1r = T("s1r")  # relu(ps1)
        nc.vector.tensor_scalar_max(out=s1r, in0=ps1, scalar1=0.0)
        s2s = T("s2s")  # relu(ps2)/pi
        nc.vector.tensor_scalar(
            out=s2s, in0=ps2, scalar1=0.0, scalar2=float(1.0 / np.pi),
            op0=OP.max, op1=OP.mult,
        )
        amp = T("amp")  # |z|*s1
        nc.vector.tensor_mul(out=amp, in0=amp0, in1=s1r)
        at = T("at")  # angle/2 (signed) = a*g1 + g0
        nc.vector.tensor_mul(out=at, in0=a, in1=g1)
        nc.vector.tensor_add(out=at, in0=at, in1=g0)
        # Nyquist: angle is exactly 0 (Re>0) or pi: at = pi/2 * (1 - (Re>0))
        nc.vector.tensor_scalar(
            out=at[:, NYQ : NYQ + 1], in0=fpn,
            scalar1=float(-np.pi / 2), scalar2=float(np.pi / 2),
            op0=OP.mult, op1=OP.add,
        )
        r_ = T("r_")  # pha / 2pi
        nc.vector.tensor_mul(out=r_, in0=at, in1=s2s)
        # sin branch first (irfft consumes ri first), cos follows
        n1 = T("n1")
        nc.vector.tensor_scalar(
            out=n1, in0=r_, scalar1=MAGIC, scalar2=MAGIC, op0=OP.add, op1=OP.subtract
        )
        nc.vector.tensor_sub(out=n1, in0=r_, in1=n1)
        sn = T("sn")
        nc.scalar.activation(out=sn, in_=n1, func=AF.Sin, scale=float(2 * np.pi))
        # cos arg from the sin arg: frac2 = (frac1 + 0.25) - (frac1 > 0.25)
        # stays in [-0.5, 0.5] (fillers while ACT runs the first sin)
        q4 = T("q4")
        nc.vector.tensor_scalar(
            out=q4, in0=n1, scalar1=0.25, scalar2=None, op0=OP.is_gt
        )
        n2 = T("n2")
        nc.vector.scalar_tensor_tensor(
            out=n2, in0=n1, scalar=0.25, in1=q4, op0=OP.add, op1=OP.subtract
        )
        cs = T("cs")
        nc.scalar.activation(out=cs, in_=n2, func=AF.Sin, scale=float(2 * np.pi))
        ri = T("ri", dt=FP16)
        nc.vector.tensor_mul(out=ri, in0=amp, in1=sn)
        rr = T("rr", dt=FP16)
        nc.vector.tensor_mul(out=rr, in0=amp, in1=cs)

        # ---- irfft as 4 fp16 matmuls into [2, C]; rec transposed into
        # [bins, 2] fp16 columns first ----
        recb = {}
        for nm, src in (("ri", ri), ("rr", rr)):
            for j, (s, l) in enumerate(FC):
                rT = ps_mm.tile([l, BPC], FP16, tag="mm", name=f"{nm}T{j}")
                nc.tensor.transpose(rT, src[:, s : s + l], id2h)
                rb = T(f"{nm}b{j}", (l, BPC), FP16)
                nc.vector.tensor_copy(out=rb, in_=rT)
                recb[(nm, j)] = rb
        pfin = ps_fin.tile([BPC, C], FP32, tag="pfin", name="pfin")
        steps = [
            (recb[("ri", 0)], ici_sb[0]), (recb[("rr", 0)], icr_sb[0]),
            (recb[("ri", 1)], ici_sb[1]), (recb[("rr", 1)], icr_sb[1]),
        ]
        for idx, (vt, mt) in enumerate(steps):
            nc.tensor.matmul(
                pfin, lhsT=vt, rhs=mt,
                start=(idx == 0), stop=(idx == len(steps) - 1),
            )
        out_sb = T("out_sb", (BPC, C))
        nc.vector.tensor_mul(out=out_sb, in0=pfin, in1=y_row)
        base = outp.ap()
        dst = bass.AP(tensor=base.tensor, offset=0, ap=[[C, BPC], [1, C]])
        nc.sync.dma_start(out=dst, in_=out_sb)

    nc.compile()
    return nc


_CACHE = {}


def _get_nc():
    if "nc" not in _CACHE:
        _CACHE["nc"] = _build()
    return _CACHE["nc"]


def _host_prep(inputs):
    import ml_dtypes

    f32, f16 = np.float32, np.float16
    bf16 = ml_dtypes.bfloat16
    W1 = np.asarray(inputs["W1"], f32)
    W2 = np.asarray(inputs["W2"], f32)
    Ws1 = np.asarray(inputs["Ws1"], f32)
    Ws2 = np.asarray(inputs["Ws2"], f32)
    b1 = np.asarray(inputs["b1"], f32)
    b2 = np.asarray(inputs["b2"], f32)
    bs1 = np.asarray(inputs["bs1"], f32)
    bs2 = np.asarray(inputs["bs2"], f32)
    # center taps of the 3x3 convs; fold the 1/HW mean scale into W1
    w1t = (W1[:, :, 1, 1].T.astype(np.float64) / HW).astype(f32)   # [C, CH]
    w2r = np.concatenate(
        [np.ascontiguousarray(W2[:, :, 1, 1].T), b2.reshape(1, C)], axis=0
    )                                                              # [CH+1, C]
    ws1t = np.ascontiguousarray(Ws1.T)                             # [C, CF]
    ws2t = np.ascontiguousarray(Ws2.T)

    i = np.arange(C, dtype=np.float64)[:, None]
    k = np.arange(CF, dtype=np.float64)[None, :]
    ang = 2.0 * np.pi * i * k / C
    cmat = np.cos(ang).astype(f32)                                 # [C, CF]
    smat = (-np.sin(ang)).astype(f32)

    kk = np.arange(CF, dtype=np.float64)[:, None]
    n = np.arange(C, dtype=np.float64)[None, :]
    ang2 = 2.0 * np.pi * kk * n / C
    alpha = np.full((CF, 1), 2.0)
    alpha[0, 0] = 1.0
    alpha[NYQ, 0] = 1.0
    icrm = (alpha * np.cos(ang2) / C).astype(f32)                  # [CF, C]
    icim = (-alpha * np.sin(ang2) / C).astype(f32)

    wu = np.zeros((128, TOTU), np.uint16)

    def put16(name, arr, dt):  # arr: [rows, cols] fp32
        o = _OFF[name]
        wu[: arr.shape[0], o : o + arr.shape[1]] = (
            arr.astype(dt).view(np.uint16)
        )

    for k3 in range(3):
        put16(f"w1t{k3}", w1t[k3 * 128 : (k3 + 1) * 128, :], bf16)
    put16("w2r", w2r, f16)
    csm = np.concatenate([cmat, smat], axis=1)                     # [C, 2CF]
    w12 = np.concatenate([ws1t, ws2t], axis=1)
    for nm, mat in (("csm", csm), ("w12", w12)):
        for k3 in range(3):
            put16(f"{nm}{k3}", mat[k3 * 128 : (k3 + 1) * 128, :], f16)
    put16("bsrow", np.concatenate([bs1, bs2]).reshape(1, 2 * CF), f16)
    put16("id2h", np.eye(BPC, dtype=f32), f16)
    for j, (s, l) in enumerate(FC):
        put16(f"icr{j}", icrm[s : s + l, :], f16)
        put16(f"ici{j}", icim[s : s + l, :], f16)
    put16("ones2", np.ones((1, BPC), f32), f16)

    wfp = np.zeros((128, TOTF), f32)
    wfp[:CH, 0] = b1
    wfp[:BPC, 1 : 1 + BPC] = np.eye(BPC, dtype=f32)
    return {"wu": wu, "wf": wfp}


def kernel(**inputs):
    x = np.asarray(inputs["x"], np.float32)
    base = _host_prep(inputs)
    nc = _get_nc()
    in_maps = [
        dict(base, xs=np.ascontiguousarray(x[i * BPC : (i + 1) * BPC]))
        for i in range(NCORES)
    ]
    res = run_bass_kernel_spmd(nc, in_maps, list(range(NCORES))).results
    return np.concatenate([res[i]["out"] for i in range(NCORES)], axis=0)
